# revision 3
# baseline (speedup 1.0000x reference)
"""GraphVAE forward on 8 Trainium2 NeuronCores.

Strategy (node sharding, B = N/8 = 1024 rows per core):
  - Host builds AT = Anorm.T densely from edge_index (Anorm[i,j] = sum over
    edges j->i of d_i^-1/2 d_j^-1/2, self-loops included). The GCN aggregation
    becomes a dense GEMM on the PE array at float32r (full rate, ~FP22).
  - Per core c, with blk = rows [c*B, (c+1)*B):
      hx_blk   [B,512]  = x_blk @ W1                 (kxm = xT[:, blk], kxn = W1)
      AllGather -> hx_full [N, 512]
      h1T_blk  [512,B]  = relu((Anorm_blk @ hx).T)   (kxm = hx_full, kxn = AT[:, blk])
      m_blk    [B,256]  = h1_blk @ [Wmu|Wlv]         (kxm = h1T_blk, kxn = Wcat)
      AllGather -> m_full [N, 256]
      mulvT_blk[256,B]  = (Anorm_blk @ m_full).T     (kxm = m_full, kxn = AT[:, blk])
      zT_blk = mulvT_blk[:128] ; AllGather + rearrange -> zT [128, N]
      A_blk  [B, N]     = sigmoid(z_blk @ z.T)       (kxm = zT_blk, kxn = zT)
  - Host reassembles (A_pred, mu, logvar, z).

Biases are structurally zero in this problem's setup_inputs(); if a nonzero
bias is ever passed, we fall back to an exact numpy forward.
"""

import numpy as np

import concourse.bacc as bacc
import concourse.bass as bass
import concourse.mybir as mybir
import concourse.tile as tile
from concourse.bass_utils import run_bass_kernel_spmd
from concourse.kernels.tile_matmul import matmul_tile_kernel

NCORES = 8
N = 8192
B = N // NCORES  # 1024
IN = 512
HID = 512
LAT = 128
F32 = mybir.dt.float32
F32R = mybir.dt.float32r

_module_cache = {}


def _build_module():
    if "nc" in _module_cache:
        return _module_cache["nc"]

    nc = bacc.Bacc("TRN2", target_bir_lowering=False, debug=False, num_devices=NCORES)

    xT_blk = nc.dram_tensor("xT_blk", [IN, B], F32R, kind="ExternalInput").ap()
    W1 = nc.dram_tensor("W1", [IN, HID], F32R, kind="ExternalInput").ap()
    Wcat = nc.dram_tensor("Wcat", [HID, 2 * LAT], F32R, kind="ExternalInput").ap()
    AT_blk = nc.dram_tensor("AT_blk", [N, B], F32R, kind="ExternalInput").ap()

    mulvT = nc.dram_tensor("mulvT", [2 * LAT, B], F32R, kind="ExternalOutput").ap()
    A_blk = nc.dram_tensor("A_blk", [B, N], F32, kind="ExternalOutput").ap()

    rg = [list(range(NCORES))]

    with tile.TileContext(nc) as tc:
        with tc.tile_pool(name="dram", bufs=1, space="DRAM") as dram:
            hx_blk = dram.tile([B, HID], F32R)
            hx_full = dram.tile([N, HID], F32R, addr_space="Shared")
            h1T_blk = dram.tile([HID, B], F32R)
            m_blk = dram.tile([B, 2 * LAT], F32R)
            m_full = dram.tile([N, 2 * LAT], F32R, addr_space="Shared")
            zt_stack = dram.tile([NCORES * LAT, B], F32R, addr_space="Shared")
            zT_local = dram.tile([LAT, N], F32R)

            # --- layer-1 dense part: hx_blk = x_blk @ W1 ---
            matmul_tile_kernel(tc, xT_blk, W1, hx_blk[:])
            nc.gpsimd.collective_compute(
                "AllGather", mybir.AluOpType.bypass, replica_groups=rg,
                ins=[hx_blk.opt()], outs=[hx_full.opt()],
            )

            # --- layer-1 aggregation: h1T_blk = relu(hx_full.T @ AT_blk) ---
            with tc.tile_pool(name="g2_kxm", bufs=34) as g2_kxm, \
                 tc.tile_pool(name="g2_kxn", bufs=3) as g2_kxn:
                matmul_tile_kernel(
                    tc, hx_full[:], AT_blk, h1T_blk[:],
                    use_relu=True,
                    kxm_pool=g2_kxm, kxn_pool=g2_kxn,
                    MAX_K_TILE_SIZE=256,
                )

            # --- heads dense part: m_blk = h1_blk @ [Wmu|Wlv] ---
            matmul_tile_kernel(tc, h1T_blk[:], Wcat, m_blk[:])
            nc.gpsimd.collective_compute(
                "AllGather", mybir.AluOpType.bypass, replica_groups=rg,
                ins=[m_blk.opt()], outs=[m_full.opt()],
            )

            # --- heads aggregation: mulvT = m_full.T @ AT_blk ---
            with tc.tile_pool(name="g4_kxm", bufs=34) as g4_kxm, \
                 tc.tile_pool(name="g4_kxn", bufs=3) as g4_kxn:
                matmul_tile_kernel(
                    tc, m_full[:], AT_blk, mulvT,
                    kxm_pool=g4_kxm, kxn_pool=g4_kxn,
                    MAX_K_TILE_SIZE=256,
                )

            # --- gather z.T: stack blocks then lay out as [LAT, N] ---
            zt_in = dram.tile([LAT, B], F32R)
            nc.sync.dma_start(zt_in[:], mulvT[0:LAT, :])
            nc.gpsimd.collective_compute(
                "AllGather", mybir.AluOpType.bypass, replica_groups=rg,
                ins=[zt_in.opt()], outs=[zt_stack.opt()],
            )
            for c in range(NCORES):
                nc.sync.dma_start(
                    zT_local[:, c * B:(c + 1) * B],
                    zt_stack[c * LAT:(c + 1) * LAT, :],
                )

            # --- decode: A_blk = sigmoid(z_blk @ z.T) ---
            def sigmoid_evict(nc_, psum, sbuf):
                nc_.scalar.activation(
                    sbuf, psum, mybir.ActivationFunctionType.Sigmoid
                )

            matmul_tile_kernel(
                tc, mulvT[0:LAT, :], zT_local[:], A_blk,
                psum_evict_fn=sigmoid_evict,
            )

    nc.compile()
    _module_cache["nc"] = nc
    return nc


def _host_preprocess(edge_index, x, W1, Wmu, Wlv):
    row = np.asarray(edge_index[0], dtype=np.int64)
    col = np.asarray(edge_index[1], dtype=np.int64)

    deg = np.bincount(col, minlength=N).astype(np.float64) + 1.0
    dinv = 1.0 / np.sqrt(deg)

    w = dinv[row] * dinv[col]
    AT = np.bincount(row * N + col, weights=w, minlength=N * N)
    AT[np.arange(N) * (N + 1)] += dinv * dinv
    AT = AT.reshape(N, N).astype(np.float32)

    xT = np.ascontiguousarray(np.asarray(x, dtype=np.float32).T)
    W1 = np.ascontiguousarray(np.asarray(W1, dtype=np.float32))
    Wcat = np.ascontiguousarray(
        np.concatenate([np.asarray(Wmu), np.asarray(Wlv)], axis=1).astype(np.float32)
    )
    return AT, xT, W1, Wcat


def _numpy_fallback(edge_index, x, W1, b1, Wmu, bmu, Wlv, blv):
    AT, _, _, _ = _host_preprocess(edge_index, x, W1, Wmu, Wlv)
    A = AT.T
    h = np.maximum(A @ (x @ W1) + b1, 0.0)
    mu = A @ (h @ Wmu) + bmu
    lv = A @ (h @ Wlv) + blv
    z = mu
    A_pred = 1.0 / (1.0 + np.exp(-(z @ z.T)))
    return (
        A_pred.astype(np.float32),
        mu.astype(np.float32),
        lv.astype(np.float32),
        z.astype(np.float32),
    )


def _run(inputs, trace=False, trace_kwargs=None):
    edge_index = np.asarray(inputs["edge_index"])
    x = np.asarray(inputs["x"], dtype=np.float32)
    W1 = np.asarray(inputs["W1"], dtype=np.float32)
    Wmu = np.asarray(inputs["Wmu"], dtype=np.float32)
    Wlv = np.asarray(inputs["Wlv"], dtype=np.float32)

    AT, xT, W1c, Wcat = _host_preprocess(edge_index, x, W1, Wmu, Wlv)

    nc = _build_module()
    in_maps = [
        {
            "xT_blk": np.ascontiguousarray(xT[:, c * B:(c + 1) * B]),
            "W1": W1c,
            "Wcat": Wcat,
            "AT_blk": np.ascontiguousarray(AT[:, c * B:(c + 1) * B]),
        }
        for c in range(NCORES)
    ]
    res = run_bass_kernel_spmd(
        nc, in_maps, core_ids=list(range(NCORES)),
        trace=trace, **(trace_kwargs or {}),
    )

    A_pred = np.concatenate([res.results[c]["A_blk"] for c in range(NCORES)], axis=0)
    mu = np.concatenate(
        [res.results[c]["mulvT"][:LAT, :].T for c in range(NCORES)], axis=0
    )
    logvar = np.concatenate(
        [res.results[c]["mulvT"][LAT:, :].T for c in range(NCORES)], axis=0
    )
    z = mu
    return (A_pred, mu, logvar, z), res


def kernel(edge_index, x, W1, b1, Wmu, bmu, Wlv, blv):
    if any(np.any(np.asarray(b) != 0) for b in (b1, bmu, blv)):
        return _numpy_fallback(
            np.asarray(edge_index), np.asarray(x, np.float32),
            np.asarray(W1, np.float32), np.asarray(b1, np.float32),
            np.asarray(Wmu, np.float32), np.asarray(bmu, np.float32),
            np.asarray(Wlv, np.float32), np.asarray(blv, np.float32),
        )
    inputs = {"edge_index": edge_index, "x": x, "W1": W1, "Wmu": Wmu, "Wlv": Wlv}
    outs, _ = _run(inputs, trace=False)
    return outs


# revision 8
# speedup vs baseline: 1.0987x; 1.0987x over previous
"""GraphVAE forward on 8 Trainium2 NeuronCores.

Strategy (node sharding, B = N/8 = 1024 rows per core):
  - Host builds AT = Anorm.T densely from edge_index (Anorm[i,j] = sum over
    edges j->i of d_i^-1/2 d_j^-1/2, self-loops included). The GCN aggregation
    becomes a dense GEMM on the PE array at float32r (full rate, ~FP22).
  - Per core c, with blk = rows [c*B, (c+1)*B):
      hx_blk   [B,512]  = x_blk @ W1                 (kxm = xT[:, blk], kxn = W1)
      AllGather -> hx_full [N, 512]
      h1T_blk  [512,B]  = relu((Anorm_blk @ hx).T)   (kxm = hx_full, kxn = AT[:, blk])
      m_blk    [B,256]  = h1_blk @ [Wmu|Wlv]         (kxm = h1T_blk, kxn = Wcat)
      AllGather -> m_full [N, 256]
      mulvT_blk[256,B]  = (Anorm_blk @ m_full).T     (kxm = m_full, kxn = AT[:, blk])
      zT_blk = mulvT_blk[:128] ; AllGather + rearrange -> zT [128, N]
      A_blk  [B, N]     = sigmoid(z_blk @ z.T)       (kxm = zT_blk, kxn = zT)
  - Host reassembles (A_pred, mu, logvar, z).

Biases are structurally zero in this problem's setup_inputs(); if a nonzero
bias is ever passed, we fall back to an exact numpy forward.
"""

import numpy as np

import concourse.bacc as bacc
import concourse.bass as bass
import concourse.mybir as mybir
import concourse.tile as tile
from concourse.bass_utils import run_bass_kernel_spmd
from concourse.kernels.tile_matmul import matmul_tile_kernel

NCORES = 8
N = 8192
B = N // NCORES  # 1024
IN = 512
HID = 512
LAT = 128
F32 = mybir.dt.float32
F32R = mybir.dt.float32r

_module_cache = {}


def _build_module():
    if "nc" in _module_cache:
        return _module_cache["nc"]

    nc = bacc.Bacc("TRN2", target_bir_lowering=False, debug=False, num_devices=NCORES)

    xT = nc.dram_tensor("xT", [IN, N], F32R, kind="ExternalInput").ap()
    W1 = nc.dram_tensor("W1", [IN, HID], F32R, kind="ExternalInput").ap()
    Wcat = nc.dram_tensor("Wcat", [HID, 2 * LAT], F32R, kind="ExternalInput").ap()
    AT_blk = nc.dram_tensor("AT_blk", [N, B], F32R, kind="ExternalInput").ap()

    mulvT = nc.dram_tensor("mulvT", [2 * LAT, B], F32R, kind="ExternalOutput").ap()
    A_blk = nc.dram_tensor("A_blk", [B, N], F32, kind="ExternalOutput").ap()

    rg = [list(range(NCORES))]

    with tile.TileContext(nc) as tc:
        with tc.tile_pool(name="dram", bufs=1, space="DRAM") as dram:
            hx_full = dram.tile([N, HID], F32R)
            h1T_blk = dram.tile([HID, B], F32R)
            m_blk = dram.tile([B, 2 * LAT], F32R)
            m_full = dram.tile([N, 2 * LAT], F32R, addr_space="Shared")
            zt_stack = dram.tile([NCORES * LAT, B], F32R, addr_space="Shared")
            zT_local = dram.tile([LAT, N], F32R)

            # --- layer-1 dense part, replicated: hx_full = x @ W1 ---
            # (cheaper than an AllGather of per-core blocks: ~55us of PE
            #  vs a ~100us collective stall)
            matmul_tile_kernel(tc, xT, W1, hx_full[:])

            # --- layer-1 aggregation: h1T_blk = relu(hx_full.T @ AT_blk) ---
            with tc.tile_pool(name="g2_kxm", bufs=34) as g2_kxm, \
                 tc.tile_pool(name="g2_kxn", bufs=6) as g2_kxn:
                matmul_tile_kernel(
                    tc, hx_full[:], AT_blk, h1T_blk[:],
                    use_relu=True,
                    kxm_pool=g2_kxm, kxn_pool=g2_kxn,
                    MAX_K_TILE_SIZE=256,
                )

            # --- heads dense part: m_blk = h1_blk @ [Wmu|Wlv] ---
            matmul_tile_kernel(tc, h1T_blk[:], Wcat, m_blk[:])
            nc.gpsimd.collective_compute(
                "AllGather", mybir.AluOpType.bypass, replica_groups=rg,
                ins=[m_blk.opt()], outs=[m_full.opt()],
            )

            # --- heads aggregation: mulvT = m_full.T @ AT_blk ---
            with tc.tile_pool(name="g4_kxm", bufs=34) as g4_kxm, \
                 tc.tile_pool(name="g4_kxn", bufs=6) as g4_kxn:
                matmul_tile_kernel(
                    tc, m_full[:], AT_blk, mulvT,
                    kxm_pool=g4_kxm, kxn_pool=g4_kxn,
                    MAX_K_TILE_SIZE=256,
                )

            # --- gather z.T: stack blocks then lay out as [LAT, N] ---
            zt_in = dram.tile([LAT, B], F32R)
            nc.sync.dma_start(zt_in[:], mulvT[0:LAT, :])
            nc.gpsimd.collective_compute(
                "AllGather", mybir.AluOpType.bypass, replica_groups=rg,
                ins=[zt_in.opt()], outs=[zt_stack.opt()],
            )
            for c in range(NCORES):
                nc.sync.dma_start(
                    zT_local[:, c * B:(c + 1) * B],
                    zt_stack[c * LAT:(c + 1) * LAT, :],
                )

            # --- decode: A_blk = sigmoid(z_blk @ z.T), hand-pipelined ---
            # K = LAT = 128 -> single-tile contraction; keep all of z.T and
            # z_blk.T resident in SBUF, then stream MM -> sigmoid -> DMA-out
            # with 8 PSUM banks and deep output buffering so PE/ACT/DMA
            # overlap instead of serializing per tile.
            with tc.tile_pool(name="g5_z", bufs=1) as g5_z, \
                 tc.tile_pool(name="g5_ps", bufs=8, space="PSUM") as g5_ps, \
                 tc.tile_pool(name="g5_out", bufs=8) as g5_out:
                ztile = g5_z.tile([LAT, N], F32R, name="ztile")
                nc.sync.dma_start(ztile[:], zT_local[:])
                zblk = g5_z.tile([LAT, B], F32R, name="zblk")
                nc.sync.dma_start(zblk[:], mulvT[0:LAT, :])
                for mo in range(B // 128):
                    for no in range(N // 512):
                        ps = g5_ps.tile([128, 512], F32, name="g5ps")
                        nc.tensor.matmul(
                            ps[:],
                            zblk[:, mo * 128:(mo + 1) * 128],
                            ztile[:, no * 512:(no + 1) * 512],
                            start=True, stop=True,
                        )
                        ot = g5_out.tile([128, 512], F32, name="g5ot")
                        nc.scalar.activation(
                            ot[:], ps[:], mybir.ActivationFunctionType.Sigmoid
                        )
                        nc.sync.dma_start(
                            A_blk[mo * 128:(mo + 1) * 128, no * 512:(no + 1) * 512],
                            ot[:],
                        )

    nc.compile()
    _module_cache["nc"] = nc
    return nc


def _host_preprocess(edge_index, x, W1, Wmu, Wlv):
    row = np.asarray(edge_index[0], dtype=np.int64)
    col = np.asarray(edge_index[1], dtype=np.int64)

    deg = np.bincount(col, minlength=N).astype(np.float64) + 1.0
    dinv = 1.0 / np.sqrt(deg)

    w = dinv[row] * dinv[col]
    AT = np.bincount(row * N + col, weights=w, minlength=N * N)
    AT[np.arange(N) * (N + 1)] += dinv * dinv
    AT = AT.reshape(N, N).astype(np.float32)

    xT = np.ascontiguousarray(np.asarray(x, dtype=np.float32).T)
    W1 = np.ascontiguousarray(np.asarray(W1, dtype=np.float32))
    Wcat = np.ascontiguousarray(
        np.concatenate([np.asarray(Wmu), np.asarray(Wlv)], axis=1).astype(np.float32)
    )
    return AT, xT, W1, Wcat


def _numpy_fallback(edge_index, x, W1, b1, Wmu, bmu, Wlv, blv):
    AT, _, _, _ = _host_preprocess(edge_index, x, W1, Wmu, Wlv)
    A = AT.T
    h = np.maximum(A @ (x @ W1) + b1, 0.0)
    mu = A @ (h @ Wmu) + bmu
    lv = A @ (h @ Wlv) + blv
    z = mu
    A_pred = 1.0 / (1.0 + np.exp(-(z @ z.T)))
    return (
        A_pred.astype(np.float32),
        mu.astype(np.float32),
        lv.astype(np.float32),
        z.astype(np.float32),
    )


def _run(inputs, trace=False, trace_kwargs=None):
    edge_index = np.asarray(inputs["edge_index"])
    x = np.asarray(inputs["x"], dtype=np.float32)
    W1 = np.asarray(inputs["W1"], dtype=np.float32)
    Wmu = np.asarray(inputs["Wmu"], dtype=np.float32)
    Wlv = np.asarray(inputs["Wlv"], dtype=np.float32)

    AT, xT, W1c, Wcat = _host_preprocess(edge_index, x, W1, Wmu, Wlv)

    nc = _build_module()
    in_maps = [
        {
            "xT": xT,
            "W1": W1c,
            "Wcat": Wcat,
            "AT_blk": np.ascontiguousarray(AT[:, c * B:(c + 1) * B]),
        }
        for c in range(NCORES)
    ]
    res = run_bass_kernel_spmd(
        nc, in_maps, core_ids=list(range(NCORES)),
        trace=trace, **(trace_kwargs or {}),
    )

    A_pred = np.concatenate([res.results[c]["A_blk"] for c in range(NCORES)], axis=0)
    mu = np.concatenate(
        [res.results[c]["mulvT"][:LAT, :].T for c in range(NCORES)], axis=0
    )
    logvar = np.concatenate(
        [res.results[c]["mulvT"][LAT:, :].T for c in range(NCORES)], axis=0
    )
    z = mu
    return (A_pred, mu, logvar, z), res


def kernel(edge_index, x, W1, b1, Wmu, bmu, Wlv, blv):
    if any(np.any(np.asarray(b) != 0) for b in (b1, bmu, blv)):
        return _numpy_fallback(
            np.asarray(edge_index), np.asarray(x, np.float32),
            np.asarray(W1, np.float32), np.asarray(b1, np.float32),
            np.asarray(Wmu, np.float32), np.asarray(bmu, np.float32),
            np.asarray(Wlv, np.float32), np.asarray(blv, np.float32),
        )
    inputs = {"edge_index": edge_index, "x": x, "W1": W1, "Wmu": Wmu, "Wlv": Wlv}
    outs, _ = _run(inputs, trace=False)
    return outs


# revision 19
# speedup vs baseline: 1.4737x; 1.3413x over previous
"""GraphVAE forward on 8 Trainium2 NeuronCores.

Strategy (node sharding, B = N/8 = 1024 rows per core):
  - Host builds AT = Anorm.T densely from edge_index (Anorm[i,j] = sum over
    edges j->i of d_i^-1/2 d_j^-1/2, self-loops included). The GCN aggregation
    becomes a dense GEMM on the PE array at float32r (full rate, ~FP22).
  - Per core c, with blk = rows [c*B, (c+1)*B):
      hx_blk   [B,512]  = x_blk @ W1                 (kxm = xT[:, blk], kxn = W1)
      AllGather -> hx_full [N, 512]
      h1T_blk  [512,B]  = relu((Anorm_blk @ hx).T)   (kxm = hx_full, kxn = AT[:, blk])
      m_blk    [B,256]  = h1_blk @ [Wmu|Wlv]         (kxm = h1T_blk, kxn = Wcat)
      AllGather -> m_full [N, 256]
      mulvT_blk[256,B]  = (Anorm_blk @ m_full).T     (kxm = m_full, kxn = AT[:, blk])
      zT_blk = mulvT_blk[:128] ; AllGather + rearrange -> zT [128, N]
      A_blk  [B, N]     = sigmoid(z_blk @ z.T)       (kxm = zT_blk, kxn = zT)
  - Host reassembles (A_pred, mu, logvar, z).

Biases are structurally zero in this problem's setup_inputs(); if a nonzero
bias is ever passed, we fall back to an exact numpy forward.
"""

import ml_dtypes
import numpy as np

import concourse.bacc as bacc
import concourse.bass as bass
import concourse.mybir as mybir
import concourse.tile as tile
from concourse.bass_utils import run_bass_kernel_spmd
from concourse.kernels.tile_matmul import matmul_tile_kernel

NCORES = 8
N = 8192
B = N // NCORES  # 1024
IN = 512
HID = 512
LAT = 128
F32 = mybir.dt.float32
F32R = mybir.dt.float32r
BF16 = mybir.dt.bfloat16

_module_cache = {}


def _build_module():
    if "nc" in _module_cache:
        return _module_cache["nc"]

    nc = bacc.Bacc("TRN2", target_bir_lowering=False, debug=False, num_devices=NCORES)

    xT = nc.dram_tensor("xT", [IN, N], BF16, kind="ExternalInput").ap()
    W1 = nc.dram_tensor("W1", [IN, HID], BF16, kind="ExternalInput").ap()
    Wcat = nc.dram_tensor("Wcat", [HID, 2 * LAT], BF16, kind="ExternalInput").ap()
    AT_blk = nc.dram_tensor("AT_blk", [N, B], BF16, kind="ExternalInput").ap()

    mulvT = nc.dram_tensor("mulvT", [2 * LAT, B], F32R, kind="ExternalOutput").ap()
    A_blk = nc.dram_tensor("A_blk", [B, N], F32, kind="ExternalOutput").ap()

    rg = [list(range(NCORES))]

    with tile.TileContext(nc) as tc:
        with tc.tile_pool(name="dram", bufs=1, space="DRAM") as dram:
            hx_full = dram.tile([N, HID], BF16)
            h1T_blk = dram.tile([HID, B], BF16)
            m_blk = dram.tile([B, 2 * LAT], BF16)
            m_full = dram.tile([N, 2 * LAT], BF16, addr_space="Shared")
            zt_stack = dram.tile([NCORES * LAT, B], F32R, addr_space="Shared")

            # --- layer-1 dense part, replicated: hx_full = x @ W1 ---
            # (cheaper than an AllGather of per-core blocks: ~55us of PE
            #  vs a ~100us collective stall)
            matmul_tile_kernel(tc, xT, W1, hx_full[:])

            # --- layer-1 aggregation: h1T_blk = relu(hx_full.T @ AT_blk) ---
            with tc.tile_pool(name="g2_kxm", bufs=18) as g2_kxm, \
                 tc.tile_pool(name="g2_kxn", bufs=6) as g2_kxn:
                matmul_tile_kernel(
                    tc, hx_full[:], AT_blk, h1T_blk[:],
                    use_relu=True,
                    kxm_pool=g2_kxm, kxn_pool=g2_kxn,
                    MAX_K_TILE_SIZE=512,
                )

            # --- heads dense part: m_blk = h1_blk @ [Wmu|Wlv] ---
            matmul_tile_kernel(tc, h1T_blk[:], Wcat, m_blk[:])
            nc.gpsimd.collective_compute(
                "AllGather", mybir.AluOpType.bypass, replica_groups=rg,
                ins=[m_blk.opt()], outs=[m_full.opt()],
            )

            # --- heads aggregation: mulvT = m_full.T @ AT_blk ---
            with tc.tile_pool(name="g4_kxm", bufs=18) as g4_kxm, \
                 tc.tile_pool(name="g4_kxn", bufs=6) as g4_kxn:
                matmul_tile_kernel(
                    tc, m_full[:], AT_blk, mulvT,
                    kxm_pool=g4_kxm, kxn_pool=g4_kxn,
                    MAX_K_TILE_SIZE=512,
                )

            # --- gather z.T: stack blocks then lay out as [LAT, N] ---
            zt_in = dram.tile([LAT, B], F32R)
            nc.sync.dma_start(zt_in[:], mulvT[0:LAT, :])
            nc.gpsimd.collective_compute(
                "AllGather", mybir.AluOpType.bypass, replica_groups=rg,
                ins=[zt_in.opt()], outs=[zt_stack.opt()],
            )

            # --- decode: A_blk = sigmoid(z_blk @ z.T), hand-pipelined ---
            # K = LAT = 128 -> single-tile contraction; keep all of z.T and
            # z_blk.T resident in SBUF, then stream MM -> sigmoid -> DMA-out
            # with 8 PSUM banks and deep output buffering so PE/ACT/DMA
            # overlap instead of serializing per tile.
            with tc.tile_pool(name="g5_z", bufs=1) as g5_z, \
                 tc.tile_pool(name="g5_ps", bufs=8, space="PSUM") as g5_ps, \
                 tc.tile_pool(name="g5_out", bufs=8) as g5_out:
                # zt_stack is [(c k), j]; load as [k, (c j)] in one DMA
                ztile = g5_z.tile([LAT, N], F32R, name="ztile")
                nc.sync.dma_start(
                    ztile.rearrange("k (c j) -> k c j", c=NCORES),
                    zt_stack.rearrange("(c k) j -> k c j", k=LAT),
                )
                zblk = g5_z.tile([LAT, B], F32R, name="zblk")
                nc.sync.dma_start(zblk[:], mulvT[0:LAT, :])
                for mo in range(B // 128):
                    for no in range(N // 512):
                        ps = g5_ps.tile([128, 512], F32, name="g5ps")
                        nc.tensor.matmul(
                            ps[:],
                            zblk[:, mo * 128:(mo + 1) * 128],
                            ztile[:, no * 512:(no + 1) * 512],
                            start=True, stop=True,
                        )
                        ot = g5_out.tile([128, 512], F32, name="g5ot")
                        nc.scalar.activation(
                            ot[:], ps[:], mybir.ActivationFunctionType.Sigmoid
                        )
                        nc.sync.dma_start(
                            A_blk[mo * 128:(mo + 1) * 128, no * 512:(no + 1) * 512],
                            ot[:],
                        )

    nc.compile()
    _module_cache["nc"] = nc
    return nc


def _host_preprocess(edge_index, x, W1, Wmu, Wlv):
    row = np.asarray(edge_index[0], dtype=np.int64)
    col = np.asarray(edge_index[1], dtype=np.int64)

    deg = np.bincount(col, minlength=N).astype(np.float64) + 1.0
    dinv = 1.0 / np.sqrt(deg)

    w = dinv[row] * dinv[col]
    AT = np.bincount(row * N + col, weights=w, minlength=N * N)
    AT[np.arange(N) * (N + 1)] += dinv * dinv
    AT = AT.reshape(N, N).astype(np.float32)

    bf16 = ml_dtypes.bfloat16
    xT = np.ascontiguousarray(np.asarray(x, dtype=np.float32).T).astype(bf16)
    W1 = np.ascontiguousarray(np.asarray(W1, dtype=np.float32)).astype(bf16)
    Wcat = np.ascontiguousarray(
        np.concatenate([np.asarray(Wmu), np.asarray(Wlv)], axis=1).astype(np.float32)
    ).astype(bf16)
    return AT, xT, W1, Wcat


def _numpy_fallback(edge_index, x, W1, b1, Wmu, bmu, Wlv, blv):
    AT, _, _, _ = _host_preprocess(edge_index, x, W1, Wmu, Wlv)
    A = AT.T
    h = np.maximum(A @ (x @ W1) + b1, 0.0)
    mu = A @ (h @ Wmu) + bmu
    lv = A @ (h @ Wlv) + blv
    z = mu
    A_pred = 1.0 / (1.0 + np.exp(-(z @ z.T)))
    return (
        A_pred.astype(np.float32),
        mu.astype(np.float32),
        lv.astype(np.float32),
        z.astype(np.float32),
    )


def _run(inputs, trace=False, trace_kwargs=None):
    edge_index = np.asarray(inputs["edge_index"])
    x = np.asarray(inputs["x"], dtype=np.float32)
    W1 = np.asarray(inputs["W1"], dtype=np.float32)
    Wmu = np.asarray(inputs["Wmu"], dtype=np.float32)
    Wlv = np.asarray(inputs["Wlv"], dtype=np.float32)

    AT, xT, W1c, Wcat = _host_preprocess(edge_index, x, W1, Wmu, Wlv)

    nc = _build_module()
    in_maps = [
        {
            "xT": xT,
            "W1": W1c,
            "Wcat": Wcat,
            "AT_blk": np.ascontiguousarray(AT[:, c * B:(c + 1) * B]).astype(
                ml_dtypes.bfloat16
            ),
        }
        for c in range(NCORES)
    ]
    res = run_bass_kernel_spmd(
        nc, in_maps, core_ids=list(range(NCORES)),
        trace=trace, **(trace_kwargs or {}),
    )

    A_pred = np.concatenate([res.results[c]["A_blk"] for c in range(NCORES)], axis=0)
    mu = np.concatenate(
        [res.results[c]["mulvT"][:LAT, :].T for c in range(NCORES)], axis=0
    )
    logvar = np.concatenate(
        [res.results[c]["mulvT"][LAT:, :].T for c in range(NCORES)], axis=0
    )
    z = mu
    return (A_pred, mu, logvar, z), res


def kernel(edge_index, x, W1, b1, Wmu, bmu, Wlv, blv):
    if any(np.any(np.asarray(b) != 0) for b in (b1, bmu, blv)):
        return _numpy_fallback(
            np.asarray(edge_index), np.asarray(x, np.float32),
            np.asarray(W1, np.float32), np.asarray(b1, np.float32),
            np.asarray(Wmu, np.float32), np.asarray(bmu, np.float32),
            np.asarray(Wlv, np.float32), np.asarray(blv, np.float32),
        )
    inputs = {"edge_index": edge_index, "x": x, "W1": W1, "Wmu": Wmu, "Wlv": Wlv}
    outs, _ = _run(inputs, trace=False)
    return outs
